# revision 13
# baseline (speedup 1.0000x reference)
"""Local window attention (B=2,T=4096,D=2048,H=16,HK=4,HD=128,WIN=512) on 8 trn2 cores.

Sequence-parallel: each core owns 1024 tokens (2 window blocks) of one batch,
plus a 512-token halo of x for computing the previous block's K/V locally.
All matmuls in bf16 (fp32 matmul is 4 cyc/row on trn2 PE; bf16 is 1), fp32
PSUM accumulation, fp32 output. No collectives; output gather is a concat.
"""

import os
import sys

for _p in ("/opt/trn_rl_repo", "/root/.axon_site/_ro/trn_rl_repo"):
    if os.path.isdir(_p) and _p not in sys.path:
        sys.path.insert(0, _p)

import ml_dtypes
import numpy as np

import concourse.bass as bass
import concourse.mybir as mybir
from concourse import bacc, bass_utils
from concourse.tile import TileContext

BF16 = mybir.dt.bfloat16
F32 = mybir.dt.float32
EXP = mybir.ActivationFunctionType.Exp

B, T, D = 2, 4096, 2048
H, HK, HD, WIN = 16, 4, 128, 512
NREP = H // HK  # 4 query heads per kv head
NCORES = 8
TOK = (B * T) // NCORES  # 1024 query tokens per core
HALO = WIN  # 512
TKV = TOK + HALO  # 1536 tokens of x for K/V
NB = TOK // WIN  # 2 query blocks per core
DC = D // 128  # 16 contraction chunks
KVW = HK * HD  # 512 (width of Wk / Wv)


def build_nc(phases: str = "qkva o", reps: int = 1) -> bass.Bass:
    # Bacc (not raw Bass): its compile() pipeline legalizes TRN2's
    # 1-sync-wait-per-instruction constraint (generate_event_semaphores),
    # which Tile-emitted multi-wait instructions otherwise violate in walrus.
    # reps>1 unrolls the whole body back-to-back inside the NEFF; used only
    # for timing (per-execution slope cancels the host dispatch overhead).
    nc = bacc.Bacc("TRN2")
    xT = nc.dram_tensor("xT", [D, TKV], BF16, kind="ExternalInput")
    wq = nc.dram_tensor("wq", [D, D], BF16, kind="ExternalInput")
    wkv = nc.dram_tensor("wkv", [D, 2 * KVW], BF16, kind="ExternalInput")
    wo = nc.dram_tensor("wo", [D, D], BF16, kind="ExternalInput")
    msk = nc.dram_tensor("msk", [2, 8, 128, WIN], BF16, kind="ExternalInput")
    out = nc.dram_tensor("out", [TOK, D], F32, kind="ExternalOutput")

    with TileContext(nc) as tc:
      for _rep in range(reps):
        with tc.tile_pool(name="pp", bufs=1) as pp:
            ones = pp.tile([128, 128], BF16, name="ones")
            nc.any.memset(ones, 1.0)

            with tc.tile_pool(name="qkv", bufs=1) as qkv:
                # QT[h]: q^T per head [d=128, tok]; after attention of block b,
                # QT[h][:, b*WIN:(b+1)*WIN] is overwritten with the normalized
                # attention output (q^T is dead by then) and feeds the O-proj.
                QT = [qkv.tile([128, TOK], BF16, name=f"qt{h}") for h in range(H)]
                KT = [qkv.tile([128, TKV], BF16, name=f"kt{g}") for g in range(HK)]
                V = [qkv.tile([128, KVW], BF16, name=f"v{t}") for t in range(TKV // 128)]
                MSK = qkv.tile([128, 2 * 8 * WIN], BF16, name="mskt")
                for v in range(2):
                    for kc in range(8):
                        j = v * 8 + kc
                        nc.sync.dma_start(MSK[:, j * WIN : (j + 1) * WIN], msk[v, kc])

                with tc.tile_pool(name="xp", bufs=1) as xp:
                    xTs = [xp.tile([128, TKV], BF16, name=f"xt{dc}") for dc in range(DC)]

                    # ---- Q^T = (x @ Wq)^T, per head, queries only ----
                    with tc.tile_pool(name="wqp", bufs=1) as wqp, \
                         tc.tile_pool(name="ps1", bufs=6, space="PSUM") as ps1:
                        wqs = [wqp.tile([128, D], BF16, name=f"wq{dc}") for dc in range(DC)]
                        for dc in range(DC):
                            nc.sync.dma_start(xTs[dc], xT[dc * 128 : (dc + 1) * 128, :])
                            nc.sync.dma_start(wqs[dc], wq[dc * 128 : (dc + 1) * 128, :])
                        for h in range(H if "q" in phases else 0):
                            for t in range(NB):
                                acc = ps1.tile([128, WIN], F32, tag="acc", name="acc")
                                for dc in range(DC):
                                    nc.tensor.matmul(
                                        acc,
                                        wqs[dc][:, h * 128 : (h + 1) * 128],
                                        xTs[dc][:, HALO + t * WIN : HALO + (t + 1) * WIN],
                                        start=(dc == 0),
                                        stop=(dc == DC - 1),
                                    )
                                nc.vector.tensor_copy(QT[h][:, t * WIN : (t + 1) * WIN], acc)

                    # ---- K^T per kv head (halo+own), V natural layout ----
                    with tc.tile_pool(name="wkvp", bufs=1) as wkvp, \
                         tc.tile_pool(name="ps2", bufs=4, space="PSUM") as ps2:
                        wkvs = [wkvp.tile([128, 2 * KVW], BF16, name=f"wkv{dc}") for dc in range(DC)]
                        for dc in range(DC):
                            nc.sync.dma_start(wkvs[dc], wkv[dc * 128 : (dc + 1) * 128, :])
                        for g in range(HK if "k" in phases else 0):
                            for t in range(TKV // WIN):
                                acc2 = ps2.tile([128, WIN], F32, tag="acc2", name="acc2")
                                for dc in range(DC):
                                    nc.tensor.matmul(
                                        acc2,
                                        wkvs[dc][:, g * 128 : (g + 1) * 128],
                                        xTs[dc][:, t * WIN : (t + 1) * WIN],
                                        start=(dc == 0),
                                        stop=(dc == DC - 1),
                                    )
                                nc.vector.tensor_copy(KT[g][:, t * WIN : (t + 1) * WIN], acc2)
                        for t in range(TKV // 128 if "v" in phases else 0):
                            accv = ps2.tile([128, KVW], F32, tag="accv", name="accv")
                            for dc in range(DC):
                                nc.tensor.matmul(
                                    accv,
                                    xTs[dc][:, t * 128 : (t + 1) * 128],
                                    wkvs[dc][:, KVW : 2 * KVW],
                                    start=(dc == 0),
                                    stop=(dc == DC - 1),
                                )
                            nc.vector.tensor_copy(V[t], accv)

                # ---- windowed attention, probs kept in [key, query] layout ----
                # scoresT chunk = KT_chunk.T @ QT  -> [kpos=128, qpos<=512]
                # p = exp(scoresT) * mask ; out^T += V_chunk.T @ p ;
                # denom (broadcast over all 128 partitions) += ones.T @ p
                # wop/osb hoisted above the attention pools so the Wo DMAs
                # prefetch during attention instead of waiting for the
                # attention pools' SBUF region to free.
                with tc.tile_pool(name="wop", bufs=1) as wop, \
                     tc.tile_pool(name="osb", bufs=3) as osb:
                  # Persistent wo tiles (one [128, D] per head, DMA'd up front)
                  # so the O-projection can be woven into the attention
                  # pipeline without per-do reloads.
                  wos = [wop.tile([128, D], BF16, name=f"wo{h}") for h in range(H)]
                  if "o" in phases:
                      for h in range(H):
                          nc.sync.dma_start(wos[h], wo[h * 128 : (h + 1) * 128, :])
                  with tc.tile_pool(name="att", bufs=6) as att, \
                     tc.tile_pool(name="scps", bufs=3, space="PSUM") as scps, \
                     tc.tile_pool(name="avps", bufs=2, space="PSUM") as avps, \
                     tc.tile_pool(name="sups", bufs=2, space="PSUM") as sups, \
                     tc.tile_pool(name="ops", bufs=1, space="PSUM") as ops:
                    def o_group(do, qc):
                        # one O-projection output tile: out[qc, do] = sum_h ...
                        acc3 = ops.tile([128, WIN], F32, tag="acc3", name="acc3")
                        for h in range(H):
                            nc.tensor.matmul(
                                acc3,
                                QT[h][:, qc * 128 : (qc + 1) * 128],
                                wos[h][:, do * WIN : (do + 1) * WIN],
                                start=(h == 0),
                                stop=(h == H - 1),
                            )
                        ost = osb.tile([128, WIN], F32, tag="ost", name="ost")
                        nc.vector.tensor_copy(ost, acc3)
                        nc.sync.dma_start(
                            out[qc * 128 : (qc + 1) * 128, do * WIN : (do + 1) * WIN],
                            ost,
                        )

                    o_groups = [
                        (do, qc)
                        for do in range(D // WIN if "o" in phases else 0)
                        for qc in range(TOK // 128)
                    ]
                    # block-0 groups (qc < 4) become ready once block-0
                    # attention is finalized; weave them into the block-1
                    # attention pipeline to fill PE gaps (attention is
                    # ACT/DVE-paced), then flush the rest at the end.
                    o_ready = [gq for gq in o_groups if gq[1] < TOK // 256]
                    o_tail = [gq for gq in o_groups if gq[1] >= TOK // 256]
                    # kc 0..3: prev block (query prefix qpos < (kc+1)*128 can
                    # attend); kc 4..7: own block (query suffix
                    # qpos >= (kc-4)*128).  kc=4 and kc=3 are full width; kc=4
                    # goes first so start=True initializes every column.
                    # One rolling software pipeline across ALL (block, head,
                    # chunk) jobs: scores/exp/mask run LA chunk-jobs ahead of
                    # the AV/denominator matmuls so the in-order PE never waits
                    # on the PE->ACT(exp)->DVE(mask)->PE chain, including
                    # across head boundaries.
                    seq = (4, 5, 6, 7, 0, 1, 2, 3)
                    geom = []
                    for kc in seq:
                        if kc < 4:
                            geom.append((kc, 0, (kc + 1) * 128))
                        else:
                            off = (kc - 4) * 128
                            geom.append((kc, off, WIN - off))
                    jobs = []  # (b, h, i) ; i indexes seq/geom
                    for b in range(NB if "a" in phases else 0):
                        for h in range(H):
                            for i in range(len(seq)):
                                jobs.append((b, h, i))
                    LA = 3
                    b0_done = 16 * len(seq) + LA  # loop idx when block-0 heads all finalized
                    ptms = {}
                    accs = {}
                    for j in range(len(jobs) + LA):
                        if "a" in phases and j > b0_done and (j - b0_done) % 8 == 0 and o_ready:
                            o_group(*o_ready.pop(0))
                        if j < len(jobs):
                            b, h, i = jobs[j]
                            g = h // NREP
                            qs = b * WIN
                            mv = 0 if b == 0 else 1
                            kc, off, w = geom[i]
                            sc = scps.tile([128, WIN], F32, tag="sc", name="sc")
                            nc.tensor.matmul(
                                sc[:, :w],
                                KT[g][:, qs + kc * 128 : qs + (kc + 1) * 128],
                                QT[h][:, qs + off : qs + off + w],
                                start=True,
                                stop=True,
                            )
                            pt = att.tile([128, WIN], BF16, tag="pt", name="pt")
                            nc.scalar.activation(pt[:, :w], sc[:, :w], EXP)
                            ptm = att.tile([128, WIN], BF16, tag="ptm", name="ptm")
                            mj = (mv * 8 + kc) * WIN
                            nc.vector.tensor_mul(
                                ptm[:, :w], pt[:, :w], MSK[:, mj + off : mj + off + w]
                            )
                            ptms[j] = ptm
                        if j >= LA:
                            b, h, i = jobs[j - LA]
                            g = h // NREP
                            qs = b * WIN
                            kc, off, w = geom[i]
                            if i == 0:
                                accs[(b, h)] = (
                                    avps.tile([128, WIN], F32, tag="avacc", name="outp"),
                                    sups.tile([128, WIN], F32, tag="sacc", name="sacc"),
                                )
                            outp, sacc = accs[(b, h)]
                            ptm = ptms.pop(j - LA)
                            nc.tensor.matmul(
                                outp[:, off : off + w],
                                V[b * 4 + kc][:, g * 128 : (g + 1) * 128],
                                ptm[:, :w],
                                start=(i == 0),
                                stop=(i == 7),
                            )
                            nc.tensor.matmul(
                                sacc[:, off : off + w],
                                ones,
                                ptm[:, :w],
                                start=(i == 0),
                                stop=(i == 7),
                            )
                            if i == 7:
                                del accs[(b, h)]
                                sinv = att.tile([128, WIN], F32, tag="sinv", name="sinv")
                                nc.vector.reciprocal(sinv, sacc)
                                nc.vector.tensor_mul(QT[h][:, qs : qs + WIN], outp, sinv)

                    # ---- remaining O projection (block-1 + any unwoven) ----
                    for do, qc in o_ready + o_tail:
                        o_group(do, qc)
    nc.compile()
    return nc


def _masks() -> np.ndarray:
    r = np.arange(WIN)[None, :]
    kp = np.arange(128)[:, None]
    reg = np.zeros((8, 128, WIN), np.float32)
    for kc in range(4):
        reg[kc] = (r < kc * 128 + kp).astype(np.float32)
    for kc in range(4, 8):
        reg[kc] = (r >= (kc - 4) * 128 + kp).astype(np.float32)
    blk0 = reg.copy()
    blk0[:4] = 0.0
    return np.stack([blk0, reg]).astype(ml_dtypes.bfloat16)


_NC_CACHE: list = []


def kernel(x: np.ndarray, Wq: np.ndarray, Wk: np.ndarray, Wv: np.ndarray, Wo: np.ndarray) -> np.ndarray:
    if not _NC_CACHE:
        _NC_CACHE.append(build_nc())
    nc = _NC_CACHE[0]

    scale = 1.0 / np.sqrt(HD)
    wq_b = np.ascontiguousarray((Wq.astype(np.float32) * scale)).astype(ml_dtypes.bfloat16)
    wkv_b = np.ascontiguousarray(
        np.concatenate([Wk, Wv], axis=1).astype(np.float32)
    ).astype(ml_dtypes.bfloat16)
    wo_b = np.ascontiguousarray(Wo.astype(np.float32)).astype(ml_dtypes.bfloat16)
    m_first = _masks()
    m_mid = np.stack([m_first[1], m_first[1]])

    xf = np.asarray(x, dtype=np.float32).reshape(B * T, D)
    cpb = NCORES // B  # cores per batch
    in_maps = []
    for c in range(NCORES):
        t0 = c * TOK
        if c % cpb == 0:
            xkv = np.concatenate([np.zeros((HALO, D), np.float32), xf[t0 : t0 + TOK]])
        else:
            xkv = xf[t0 - HALO : t0 + TOK]
        xT_c = np.ascontiguousarray(xkv.T).astype(ml_dtypes.bfloat16)
        in_maps.append(
            {
                "xT": xT_c,
                "wq": wq_b,
                "wkv": wkv_b,
                "wo": wo_b,
                "msk": m_first if c % cpb == 0 else m_mid,
            }
        )

    res = bass_utils.run_bass_kernel_spmd(nc, in_maps, core_ids=list(range(NCORES)))
    outs = [res.results[c]["out"] for c in range(NCORES)]
    return np.concatenate(outs, axis=0).reshape(B, T, D)

